# revision 24
# baseline (speedup 1.0000x reference)
"""Trainium2 Bass kernel for AdvancedAudioStegEncoder (B=4, L=4096, 8 cores).

Sharding: hybrid batch x sequence. Core c handles batch b=c//2, half h=c%2.
h=1 cores receive x-REVERSED data + tap-flipped conv kernels from the host,
so every core's own half is the LEFT half of its oriented sequence and all
graph slice offsets are uniform (required for SPMD single-graph execution).

Key structure (vs the straightforward port):
  - conv1 is computed over ALL 4 batches' full length on every core (cheap:
    4-way row-packed matmuls, one per batch concurrently in the PE array).
    BN1 batch stats are then computed locally -> NO collective for BN1, and
    attn1 keys/values for the full 4096 positions are local -> NO AllGather.
    The BN1 mean uses sum(y1) = w1c^T @ colsum(x0) (one tiny matmul).
  - BN2: AllReduce of stats runs CONCURRENTLY with a pair-AllGather of the
    pre-BN conv2 output (y2); remote K/V for attn2 are computed locally
    after BN2 arrives. 2 collective stall windows total (BN2, BN3).
  - Attention scores matmuls (contraction d=8/16) are 4-way row-packed via
    tile_position with 4x-replicated projection weights; exp runs 2048 wide
    across the 4 score PSUM banks in one ScalarE instruction.
  - attn softmax normalizer: attn1 via a 65th ones-column of V^T; attn2 via
    ones-lhsT matmuls over pair-pre-reduced probs (DVE adds halve the
    extra PE streams). gamma is folded into the V projection weights.
  - BN rstd via exp(-0.5*ln(var)) so only one ACT table set (exp+ln) is
    ever loaded; BN apply is fused scale+bias+relu on ScalarE/VectorE.
  - Dummy matmul->copy chains keep the PE HAM clock warm through the two
    collective stalls; a dummy AllGather at kernel start absorbs core skew.
"""
import sys
import numpy as np

sys.path.insert(0, "/opt/trn_rl_repo")

import ml_dtypes

import concourse.bass as bass
import concourse.bacc as bacc
import concourse.tile as tile
import concourse.mybir as mybir
from concourse.bass_utils import run_bass_kernel_spmd

BF16 = mybir.dt.bfloat16
F32 = mybir.dt.float32
AF = mybir.ActivationFunctionType
ALU = mybir.AluOpType
AX = mybir.AxisListType

B, L = 4, 4096
NCORES = 8
OWN = 2048
MG = 32
W = OWN + 2 * MG          # 2112 query window (cols 0..2112 = pos -32..2080)
XW = 4160                 # x-full cols: [0,4128) = pos [-32,4096), + 32 pad
KEY0 = MG                 # keys = cols [32, 4128) = pos [0, 4096)
NK = 4096
PD = 8                    # zero pad around conv input tiles
WP = W + 2 * PD           # 2128
EPS = 1e-5
NSTAT = float(B * L)
STRENGTH = 0.01
JT = 32                   # key tiles of 128

# query-window chunks
CHUNKS = [(0, 512), (512, 512), (1024, 512), (1536, 512), (2048, 64)]
CPAIRS = [(0, 1), (2, 3), (4,)]
# own-real stat slices within the window
STAT_SL = [(32, 480), (512, 512), (1024, 512), (1536, 512), (2048, 32)]
# x-full chunks (9 per batch, conv1 / x1full apply)
FCHUNKS = [(i * 512, 512) for i in range(8)] + [(4096, 32)]
# stat slices for conv1 within x-full chunks (exclude cols [0,32) = pos<0)
FSTAT_SL = [(32, 480)] + [(i * 512, 512) for i in range(1, 8)] + [(4096, 32)]
OUT_CHUNKS = [(32, 512), (544, 512), (1056, 512), (1568, 512)]

LAST_RESULTS = None


def _bf(x):
    return np.ascontiguousarray(x).astype(ml_dtypes.bfloat16)


def _f32(x):
    return np.ascontiguousarray(x).astype(np.float32)


# ---------------- constant blob layout (bf16, [128, NB]) ----------------
# offsets (cols)
O_W1C = 0          # [128,128] w1c_dup per batch row-group
O_WK1 = 128        # [128,128] wk1 4x col-replicated, row-duplicated
O_WQ1 = 256
O_WV1 = 384        # [128,64] wv1^T * g1, row-duplicated
O_WK2 = 448        # [128,128] wk2 4x col-replicated (16 used + 16 zero each)
O_WQ2 = 576
O_WV2 = 704        # [128,128] wv2^T * g2
O_W2P = 832        # [128, 8*128] conv2 tap pairs
O_W3T = 1856       # [128, 15*128] conv3 taps (out-duplicated)
O_W4P = 3776       # [128, 8*64] conv4 tap pairs (out-duplicated)
O_W5P = 4288       # [64, 8] conv5 tap pairs
NB = 4296

# constf blob (f32, [128, NF])
C_BN1 = 0   # g,b dup  (cols 0,1)
C_BN2 = 2
C_BN3 = 4   # dup
C_C4B = 6   # c4b dup (rows 0:64)
NF = 8


def build_graph(c5b_f: float):
    nc = bacc.Bacc("TRN2", target_bir_lowering=False, debug=False,
                   num_devices=NCORES)

    def din(name, shape, dt=BF16):
        return nc.dram_tensor(name, shape, dt, kind="ExternalInput")

    cb_d = din("cb", [128, NB])
    cf_d = din("cf", [128, NF], F32)
    x0_d = din("x0", [128, XW])
    aud_d = din("aud", [1, OWN], F32)

    out_d = nc.dram_tensor("out", [1, OWN], F32, kind="ExternalOutput")

    # collective buffers
    dum_in = nc.dram_tensor("dum_in", [128, 2], F32, kind="Internal")
    dum_out = nc.dram_tensor("dum_out", [8, 128, 2], F32, kind="Internal",
                             addr_space="Shared")
    ar_in = [nc.dram_tensor(f"ar{i}_in", [128, 2], F32, kind="Internal")
             for i in range(2)]
    pw_in = [nc.dram_tensor(f"pw{i}_in", [64, 2], F32, kind="Internal")
             for i in range(2)]
    pw_out = [nc.dram_tensor(f"pw{i}_out", [2, 64, 2], F32, kind="Internal")
              for i in range(2)]
    ar_out = [nc.dram_tensor(f"ar{i}_out", [8, 128, 2], F32, kind="Internal",
                             addr_space="Shared") for i in range(2)]
    AGN = 128 * OWN
    ag_in = nc.dram_tensor("ag_in", [AGN], BF16, kind="Internal")
    ag_out = nc.dram_tensor("ag_out", [2, AGN], BF16, kind="Internal")

    PAIRS = [[0, 1], [2, 3], [4, 5], [6, 7]]
    ALL8 = [list(range(8))]

    with tile.TileContext(nc) as tc:
        with tc.tile_pool(name="const", bufs=1) as cp, \
             tc.tile_pool(name="act", bufs=1) as ap_, \
             tc.tile_pool(name="flow", bufs=1) as fp, \
             tc.tile_pool(name="ps", bufs=1, space="PSUM") as ps:

            # ---------------- loads ----------------
            cf = cp.tile([128, NF], F32, tag="cf")
            nc.sync.dma_start(cf[:], cf_d.ap())
            x0 = cp.tile([128, XW], BF16, tag="x0")
            nc.sync.dma_start(x0[:], x0_d.ap())
            cb = cp.tile([128, NB], BF16, tag="cb")
            nc.scalar.dma_start(cb[:], cb_d.ap())
            aud = cp.tile([1, OWN], F32, tag="aud")
            nc.sync.dma_start(aud[:], aud_d.ap())

            ones128 = cp.tile([128, 1], BF16, tag="ones128")
            nc.vector.memset(ones128[:], 1.0)

            # dummy collective to absorb initial core skew + warm CC path
            nc.sync.dma_start(dum_in.ap(), cf[:, 0:2])
            nc.gpsimd.collective_compute(
                "AllGather", ALU.bypass, replica_groups=ALL8,
                ins=[dum_in.ap().opt()], outs=[dum_out.ap().opt()])

            # psum tag helpers -------------------------------------------------
            _nid = [0]

            def sct():
                _nid[0] += 1
                return ps.tile([128, 2, 512], F32, tag="sc", bufs=2,
                               name=f"sc{_nid[0]}")

            def avt():
                _nid[0] += 1
                return ps.tile([128, 512], F32, tag="av", bufs=2,
                               name=f"av{_nid[0]}")

            def zzt():
                _nid[0] += 1
                return ps.tile([128, 512], F32, tag="zz", bufs=2,
                               name=f"zz{_nid[0]}")

            # ---------------- conv1: all 4 batches, 4-way row-packed --------
            y1own = ap_.tile([128, XW], BF16, tag="y1own")
            sq_scrA = fp.tile([128, 512], F32, tag="sq_scrA")  # ACT scratch
            sq_scrD = fp.tile([128, 512], F32, tag="sq_scrD")  # DVE scratch
            sq_scrD2 = fp.tile([128, 512], F32, tag="sq_scrD2")
            st1 = fp.tile([128, 40], F32, tag="st1")  # sumsq slots
            nslot = [0]
            for ci, (cs, cw) in enumerate(FCHUNKS):
                # alternate bank sets so stats of span n overlap span n+1
                if ci % 2 == 0:
                    spanA = sct()
                    spanB = sct()
                    banks = [spanA[:, 0, :], spanA[:, 1, :],
                             spanB[:, 0, :], spanB[:, 1, :]]
                else:
                    quad = [ps.tile([128, 512], F32, tag="av", bufs=2, name=f"cq{ci}a"),
                            ps.tile([128, 512], F32, tag="av", bufs=2, name=f"cq{ci}b"),
                            ps.tile([128, 512], F32, tag="zz", bufs=2, name=f"cq{ci}c"),
                            ps.tile([128, 512], F32, tag="zz", bufs=2, name=f"cq{ci}d")]
                    banks = [q_[0:128, :] for q_ in quad]
                for b in range(4):
                    nc.tensor.matmul(
                        banks[b][:, 0:cw],
                        cb[32 * b:32 * b + 30, O_W1C:O_W1C + 128],
                        x0[32 * b:32 * b + 30, cs:cs + cw],
                        start=True, stop=True, tile_position=(32 * b, 0))
                a, wd = FSTAT_SL[ci]
                o = a - cs
                for b in range(4):
                    sl = banks[b][:, o:o + wd]
                    slot = nslot[0]; nslot[0] += 1
                    if b != 1:
                        # ACT: square+accum straight from PSUM
                        nc.scalar.activation(sq_scrA[:, 0:wd], sl, AF.Square,
                                             accum_out=st1[:, slot:slot + 1])
                    else:
                        # DVE: copy out, square, reduce (DVE can't read two
                        # PSUM operands)
                        nc.vector.tensor_copy(sq_scrD[:, 0:wd], sl)
                        nc.vector.tensor_mul(sq_scrD2[:, 0:wd],
                                             sq_scrD[:, 0:wd],
                                             sq_scrD[:, 0:wd])
                        nc.vector.tensor_reduce(st1[:, slot:slot + 1],
                                                sq_scrD2[:, 0:wd],
                                                axis=AX.X, op=ALU.add)
                # own batch (group 0): copy out y1 (DVE; ACT is square-bound)
                nc.vector.tensor_copy(y1own[:, cs:cs + cw],
                                      banks[0][:, 0:cw])

            # mean via colsum trick: s0 = rowsum(x0 over stat cols)
            s0 = fp.tile([128, 2], F32, tag="s0")
            s0b = fp.tile([128, 1], BF16, tag="s0b")
            nc.vector.tensor_reduce(s0[:, 0:1], x0[:, 32:4128],
                                    axis=AX.X, op=ALU.add)
            nc.vector.tensor_copy(s0b[:], s0[:, 0:1])
            sum1ps = avt()
            nc.tensor.matmul(sum1ps[0:128, 0:1], cb[:, O_W1C:O_W1C + 128],
                             s0b[:], start=True, stop=True)
            # total sumsq
            ssq1 = fp.tile([128, 2], F32, tag="ssq1")
            nc.vector.tensor_reduce(ssq1[:, 0:1], st1[:, 0:36],
                                    axis=AX.X, op=ALU.add)

            def bn_scale_shift(sum_ap, sumsq_ap, gb_col, tag):
                """sum/sumsq [128,1] -> (scale, shift) [128,1] f32 via ln/exp"""
                s = fp.tile([128, 8], F32, tag=tag)
                nc.vector.tensor_scalar_mul(s[:, 0:1], sum_ap, 1.0 / NSTAT)
                nc.vector.tensor_scalar_mul(s[:, 1:2], sumsq_ap, 1.0 / NSTAT)
                nc.vector.tensor_mul(s[:, 2:3], s[:, 0:1], s[:, 0:1])
                nc.vector.tensor_sub(s[:, 2:3], s[:, 1:2], s[:, 2:3])
                nc.vector.tensor_scalar_add(s[:, 2:3], s[:, 2:3], EPS)
                # rstd = exp(-0.5*ln(var))
                nc.scalar.activation(s[:, 3:4], s[:, 2:3], AF.Ln)
                nc.scalar.activation(s[:, 4:5], s[:, 3:4], AF.Exp, scale=-0.5)
                nc.vector.tensor_mul(s[:, 5:6], s[:, 4:5],
                                     cf[:, gb_col:gb_col + 1])
                nc.vector.tensor_mul(s[:, 6:7], s[:, 0:1], s[:, 5:6])
                nc.vector.tensor_sub(s[:, 6:7], cf[:, gb_col + 1:gb_col + 2],
                                     s[:, 6:7])
                return s  # scale = s[:,5:6], shift = s[:,6:7]

            ss1 = bn_scale_shift(sum1ps[0:128, 0:1], ssq1[:, 0:1], C_BN1, "ss1")

            # ---------------- x1full = relu(bn1(y1own)), dup rows ----------
            x1f = ap_.tile([128, XW], BF16, tag="x1f")
            for ci, (cs, cw) in enumerate(FCHUNKS):
                if ci % 2 == 0:
                    nc.scalar.activation(x1f[:, cs:cs + cw], y1own[:, cs:cs + cw],
                                         AF.Relu, bias=ss1[:, 6:7],
                                         scale=ss1[:, 5:6])
                else:
                    nc.vector.tensor_scalar(x1f[:, cs:cs + cw],
                                            y1own[:, cs:cs + cw],
                                            scalar1=ss1[:, 5:6],
                                            scalar2=ss1[:, 6:7],
                                            op0=ALU.mult, op1=ALU.add)
                    nc.vector.tensor_scalar_max(x1f[:, cs:cs + cw],
                                                x1f[:, cs:cs + cw], 0.0)
            nc.vector.memset(x1f[:, 0:MG], 0.0)  # OOB query margin

            wch = [fp.tile([128, 512], BF16, tag=f"wch{i}", name=f"wch{i}")
                   for i in range(2)]
            nc.vector.memset(wch[0][:], 0.001)
            nc.vector.memset(wch[1][:], 0.001)

            def dve_warm_chain(n, gate_f32_ap):
                """Keep the PE HAM clock warm through a collective stall:
                matmul every ~1.2us, paced by DVE psum->sbuf copies."""
                nc.vector.tensor_scalar_mul(wch[0][:, 0:2], gate_f32_ap, 1e-3)
                for k in range(n):
                    t = zzt()
                    nc.tensor.matmul(t[0:128, 0:512], cb[0:128, 0:128],
                                     wch[k % 2][:], start=True, stop=True)
                    nc.vector.tensor_scalar_mul(wch[(k + 1) % 2][:],
                                                t[:, 0:512], 1e-3)

            # ---------------- attention (shared for both layers) -----------
            def attention(xq, xk, wk_off, wq_off, wv_off, d, vcols, use_zz,
                          kq_sb, q_sb, vt_sb, epilogue, mid_hook=None):
                """xq [128, W] (dup rows for attn1), xk [128, NK] keys source.
                kq/q 4x-replicated projections; vt built per 128-key tile.
                epilogue(ci, cs, cw, av_ps, rzb) writes the output chunk."""
                # k projection over keys (8 chunks of 512)
                for c2 in range(0, 8, 2):
                    sp = sct()
                    if d == 8:  # 2-way packed (K=64, dup rows)
                        for u in range(2):
                            nc.tensor.matmul(
                                sp[:, u, :],
                                cb[64 * u:64 * u + 64, wk_off:wk_off + 128],
                                xk[64 * u:64 * u + 64,
                                   512 * (c2 + u):512 * (c2 + u) + 512],
                                start=True, stop=True,
                                tile_position=(64 * u, 0))
                    else:
                        for u in range(2):
                            nc.tensor.matmul(
                                sp[:, u, :], cb[:, wk_off:wk_off + 128],
                                xk[:, 512 * (c2 + u):512 * (c2 + u) + 512],
                                start=True, stop=True)
                    if (c2 // 2) % 2 == 0:
                        nc.scalar.activation(kq_sb[:, 512 * c2:512 * (c2 + 2)],
                                             sp[:, 0:2, :], AF.Copy)
                    else:
                        nc.vector.tensor_copy(kq_sb[:, 512 * c2:512 * (c2 + 2)],
                                              sp[:, 0:2, :])
                # q projection over window (5 chunks)
                spq1 = sct()
                spq2 = sct()
                for ci in range(4):
                    spq = spq1 if ci < 2 else spq2
                    if d == 8:
                        u = ci % 2
                        nc.tensor.matmul(
                            spq[:, ci % 2, :],
                            cb[64 * u:64 * u + 64, wq_off:wq_off + 128],
                            xq[64 * u:64 * u + 64, 512 * ci:512 * ci + 512],
                            start=True, stop=True, tile_position=(64 * u, 0))
                    else:
                        nc.tensor.matmul(
                            spq[:, ci % 2, :], cb[:, wq_off:wq_off + 128],
                            xq[:, 512 * ci:512 * ci + 512],
                            start=True, stop=True)
                nc.scalar.activation(q_sb[:, 0:1024], spq1[:, 0:2, :],
                                     AF.Copy)
                nc.vector.tensor_copy(q_sb[:, 1024:2048], spq2[:, 0:2, :])
                q5 = avt()
                if d == 8:
                    nc.tensor.matmul(q5[0:128, 0:64],
                                     cb[0:64, wq_off:wq_off + 128],
                                     xq[0:64, 2048:2112],
                                     start=True, stop=True)
                else:
                    nc.tensor.matmul(q5[0:128, 0:64], cb[:, wq_off:wq_off + 128],
                                     xq[:, 2048:2112], start=True, stop=True)
                nc.vector.tensor_copy(q_sb[:, 2048:2112], q5[:, 0:64])

                # vT per key tile
                for j2 in range(0, JT, 2):
                    va = avt()
                    vb = avt()
                    if d == 8:
                        nc.tensor.matmul(va[0:128, 0:64],
                                         xk[0:64, 128 * j2:128 * j2 + 128],
                                         cb[0:64, wv_off:wv_off + 64],
                                         start=True, stop=True)
                        nc.tensor.matmul(vb[0:128, 0:64],
                                         xk[64:128, 128 * (j2 + 1):128 * (j2 + 1) + 128],
                                         cb[64:128, wv_off:wv_off + 64],
                                         start=True, stop=True,
                                         tile_position=(64, 0))
                        nc.vector.tensor_copy(vt_sb[:, j2, 0:64], va[:, 0:64])
                        nc.vector.tensor_copy(vt_sb[:, j2 + 1, 0:64], vb[:, 0:64])
                    else:
                        nc.tensor.matmul(va[0:128, 0:128],
                                         xk[:, 128 * j2:128 * j2 + 128],
                                         cb[:, wv_off:wv_off + 128],
                                         start=True, stop=True)
                        nc.tensor.matmul(vb[0:128, 0:128],
                                         xk[:, 128 * (j2 + 1):128 * (j2 + 1) + 128],
                                         cb[:, wv_off:wv_off + 128],
                                         start=True, stop=True)
                        if (j2 // 2) % 2 == 0:
                            nc.scalar.activation(vt_sb[:, j2, 0:128],
                                                 va[:, 0:128], AF.Copy)
                            nc.scalar.activation(vt_sb[:, j2 + 1, 0:128],
                                                 vb[:, 0:128], AF.Copy)
                        else:
                            nc.vector.tensor_copy(vt_sb[:, j2, 0:128],
                                                  va[:, 0:128])
                            nc.vector.tensor_copy(vt_sb[:, j2 + 1, 0:128],
                                                  vb[:, 0:128])
                if vcols == 65:
                    nc.vector.memset(vt_sb[:, :, 64:65], 1.0)

                # main loop over chunk pairs
                for cp_ in CPAIRS:
                    avps = {ci: avt() for ci in cp_}
                    zzps = {ci: zzt() for ci in cp_} if use_zz else {}
                    for jg in range(8):
                        prb = {}
                        for ci in cp_:
                            cs, cw = CHUNKS[ci]
                            p = fp.tile([128, 4, 512], BF16, tag="probs",
                                        bufs=4, name=f"p{ci}")
                            for half in range(2):
                                sp = sct()
                                for u in range(2):
                                    i = 2 * half + u
                                    j = 4 * jg + i
                                    nc.tensor.matmul(
                                        sp[:, u, 0:cw],
                                        kq_sb[32 * i:32 * i + d,
                                              128 * j:128 * j + 128],
                                        q_sb[32 * i:32 * i + d, cs:cs + cw],
                                        start=True, stop=True,
                                        tile_position=(32 * i, 0))
                                nc.scalar.activation(
                                    p[:, 2 * half:2 * half + 2, 0:cw],
                                    sp[:, :, 0:cw], AF.Exp)
                            prb[ci] = p
                        # av: i-outer so each vT tile is loaded once per pair
                        for i in range(4):
                            j = 4 * jg + i
                            for ci in cp_:
                                cs, cw = CHUNKS[ci]
                                nc.tensor.matmul(
                                    avps[ci][0:vcols, 0:cw], vt_sb[:, j, :],
                                    prb[ci][:, i, 0:cw],
                                    start=(jg == 0 and i == 0),
                                    stop=(jg == 7 and i == 3))
                        if use_zz:
                            for ci in cp_:
                                cs, cw = CHUNKS[ci]
                                p = prb[ci]
                                p01 = fp.tile([128, 512], BF16, tag="p01",
                                              bufs=4, name=f"p01_{ci}")
                                p23 = fp.tile([128, 512], BF16, tag="p23",
                                              bufs=4, name=f"p23_{ci}")
                                nc.vector.tensor_add(p01[:, 0:cw], p[:, 0, 0:cw],
                                                     p[:, 1, 0:cw])
                                nc.vector.tensor_add(p23[:, 0:cw], p[:, 2, 0:cw],
                                                     p[:, 3, 0:cw])
                                nc.tensor.matmul(zzps[ci][0:1, 0:cw], ones128[:],
                                                 p01[:, 0:cw],
                                                 start=(jg == 0), stop=False)
                                nc.tensor.matmul(zzps[ci][0:1, 0:cw], ones128[:],
                                                 p23[:, 0:cw],
                                                 start=False, stop=(jg == 7))
                    # epilogue for this chunk pair: one batched reciprocal
                    # (second Z row parked at partition 32 for AP alignment)
                    zr = fp.tile([64, 512], F32, tag="zr", bufs=2, name="zr")
                    nc.vector.memset(zr[:], 1.0)
                    for k, ci in enumerate(cp_):
                        cs, cw = CHUNKS[ci]
                        zrow = (zzps[ci][0:1, 0:cw] if use_zz
                                else avps[ci][64:65, 0:cw])
                        nc.vector.tensor_copy(zr[32 * k:32 * k + 1, 0:cw],
                                              zrow)
                    nc.vector.reciprocal(zr[0:64, :], zr[0:64, :])
                    for k, ci in enumerate(cp_):
                        cs, cw = CHUNKS[ci]
                        if k == 0:
                            zsrc = zr[0:1, 0:cw]
                        else:
                            zc = fp.tile([1, 512], F32, tag="zc", bufs=2,
                                         name="zc")
                            nc.vector.tensor_copy(zc[:, 0:cw],
                                                  zr[32:33, 0:cw])
                            zsrc = zc[:, 0:cw]
                        rzb_ = fp.tile([128, 512], F32, tag="rzb", bufs=2,
                                       name=f"rzb{ci}")
                        nc.gpsimd.partition_broadcast(rzb_[:, 0:cw], zsrc)
                        epilogue(ci, cs, cw, avps[ci], rzb_)
                    if mid_hook is not None:
                        mid_hook(cp_, zr)

            # ---------------- attn1 ----------------
            kq1 = ap_.tile([128, NK], BF16, tag="kq1")
            q1 = ap_.tile([128, W], BF16, tag="q1")
            vt1 = ap_.tile([128, JT, 65], BF16, tag="vt1")
            x1ad = ap_.tile([128, WP], BF16, tag="x1ad")
            nc.vector.memset(x1ad[:, 0:PD], 0.0)
            nc.vector.memset(x1ad[:, PD + W - 1:WP], 0.0)
            atmp = fp.tile([64, 512], F32, tag="atmp", bufs=2)

            def epi1(ci, cs, cw, avp, rzb):
                nc.vector.tensor_mul(atmp[:, 0:cw], avp[0:64, 0:cw],
                                     rzb[0:64, 0:cw])
                nc.vector.tensor_add(x1ad[0:64, PD + cs:PD + cs + cw],
                                     atmp[:, 0:cw], x1f[0:64, cs:cs + cw])
                nc.vector.tensor_add(x1ad[64:128, PD + cs - 1:PD + cs + cw - 1],
                                     atmp[:, 0:cw], x1f[0:64, cs:cs + cw])
                if ci == 0:
                    nc.vector.memset(x1ad[0:64, PD:PD + MG], 0.0)
                    nc.vector.memset(x1ad[64:128, PD - 1:PD + MG - 1], 0.0)

            def mid1(cp_, zr_tile):
                if cp_[0] == 2:  # after chunk pair (2,3): ~2/3 through attn1
                    nc.sync.dma_start(pw_in[0].ap(), zr_tile[0:64, 0:2])
                    nc.gpsimd.collective_compute(
                        "AllGather", ALU.bypass, replica_groups=PAIRS,
                        ins=[pw_in[0].ap().opt()],
                        outs=[pw_out[0].ap().opt()])

            attention(x1f[:, 0:W], x1f[:, KEY0:KEY0 + NK], O_WK1, O_WQ1, O_WV1,
                      8, 65, False, kq1, q1, vt1, epi1, mid_hook=mid1)

            # ---------------- conv2 (tap pairs, tap-outer) + stats ---------
            y2 = ap_.tile([128, W], BF16, tag="y2")
            c2banksA = sct()
            c2banksB = sct()
            c2small = avt()

            def _cbank(tiles, small, ci, rows=128):
                ta, tb = tiles
                if ci < 2:
                    return ta[0:rows, ci, :]
                if ci < 4:
                    return tb[0:rows, ci - 2, :]
                return small[0:rows, :]
            for t in range(8):
                for ci, (cs, cw) in enumerate(CHUNKS):
                    dst = _cbank((c2banksA, c2banksB), c2small, ci)[:, 0:cw]
                    nc.tensor.matmul(dst, cb[:, O_W2P + 128 * t:O_W2P + 128 * (t + 1)],
                                     x1ad[:, PD + cs + 2 * t - 7:PD + cs + 2 * t - 7 + cw],
                                     start=(t == 0), stop=(t == 7))
            st2 = fp.tile([128, 12], F32, tag="st2")
            for ci, (cs, cw) in enumerate(CHUNKS):
                bank = _cbank((c2banksA, c2banksB), c2small, ci)
                src = bank[:, 0:cw]
                a, wd = STAT_SL[ci]
                sl = bank[:, a - cs:a - cs + wd]
                nc.vector.tensor_reduce(st2[:, ci:ci + 1], sl, axis=AX.X,
                                        op=ALU.add)
                nc.scalar.activation(sq_scrA[:, 0:wd], sl, AF.Square,
                                     accum_out=st2[:, 5 + ci:6 + ci])
                if ci % 2 == 0:
                    nc.scalar.activation(y2[:, cs:cs + cw], src, AF.Copy)
                else:
                    nc.vector.tensor_copy(y2[:, cs:cs + cw], src)
            stats2 = fp.tile([128, 2], F32, tag="stats2")
            nc.vector.tensor_reduce(stats2[:, 0:1], st2[:, 0:5], axis=AX.X,
                                    op=ALU.add)
            nc.vector.tensor_reduce(stats2[:, 1:2], st2[:, 5:10], axis=AX.X,
                                    op=ALU.add)
            nc.sync.dma_start(ar_in[0].ap(), stats2[:])
            nc.gpsimd.collective_compute(
                "AllGather", ALU.bypass, replica_groups=ALL8,
                ins=[ar_in[0].ap().opt()], outs=[ar_out[0].ap().opt()])
            # concurrent pair-AllGather of pre-BN y2 (own real region)
            nc.sync.dma_start(
                ag_in.ap().rearrange("(p c) -> p c", p=128),
                y2[:, MG:MG + OWN])
            nc.gpsimd.collective_compute(
                "AllGather", ALU.bypass, replica_groups=PAIRS,
                ins=[ag_in.ap().opt()], outs=[ag_out.ap().opt()])
            dve_warm_chain(10, stats2[0:128, 0:2])

            def bn_from_ar(ar_dram, gb_col, tag):
                s8 = fp.tile([128, 8, 2], F32, tag=tag + "g")
                nc.sync.dma_start(
                    s8[:], ar_dram.ap().rearrange("b p c -> p b c"))
                sred = fp.tile([128, 2], F32, tag=tag + "r")
                nc.vector.tensor_reduce(sred[:, 0:2],
                                        s8[:].rearrange("p b c -> p c b"),
                                        axis=AX.X, op=ALU.add)
                return bn_scale_shift(sred[:, 0:1], sred[:, 1:2], gb_col, tag)

            ss2 = bn_from_ar(ar_out[0], C_BN2, "ss2")

            # x2q (own window) and x2k (gathered pair keys)
            x2q = ap_.tile([128, W], BF16, tag="x2q")
            for ci, (cs, cw) in enumerate(CHUNKS):
                if ci % 2 == 0:
                    nc.scalar.activation(x2q[:, cs:cs + cw], y2[:, cs:cs + cw],
                                         AF.Relu, bias=ss2[:, 6:7],
                                         scale=ss2[:, 5:6])
                else:
                    nc.vector.tensor_scalar(x2q[:, cs:cs + cw],
                                            y2[:, cs:cs + cw],
                                            scalar1=ss2[:, 5:6],
                                            scalar2=ss2[:, 6:7],
                                            op0=ALU.mult, op1=ALU.add)
                    nc.vector.tensor_scalar_max(x2q[:, cs:cs + cw],
                                                x2q[:, cs:cs + cw], 0.0)
            nc.vector.memset(x2q[:, 0:MG], 0.0)
            x2kr = ap_.tile([128, NK], BF16, tag="x2kr")
            for blk in range(2):
                nc.sync.dma_start(
                    x2kr[:, OWN * blk:OWN * (blk + 1)],
                    ag_out[blk].rearrange("(p c) -> p c", p=128))
            x2k = ap_.tile([128, NK], BF16, tag="x2k")
            for c8 in range(8):
                sl = slice(512 * c8, 512 * (c8 + 1))
                if c8 % 2 == 0:
                    nc.scalar.activation(x2k[:, sl], x2kr[:, sl], AF.Relu,
                                         bias=ss2[:, 6:7], scale=ss2[:, 5:6])
                else:
                    nc.vector.tensor_scalar(x2k[:, sl], x2kr[:, sl],
                                            scalar1=ss2[:, 5:6],
                                            scalar2=ss2[:, 6:7],
                                            op0=ALU.mult, op1=ALU.add)
                    nc.vector.tensor_scalar_max(x2k[:, sl], x2k[:, sl], 0.0)

            # ---------------- attn2 ----------------
            kq2 = ap_.tile([128, NK], BF16, tag="kq2")
            q2 = ap_.tile([128, W], BF16, tag="q2")
            vt2 = ap_.tile([128, JT, 128], BF16, tag="vt2")
            x2a = ap_.tile([128, WP], BF16, tag="x2a")
            nc.vector.memset(x2a[:, 0:PD], 0.0)
            nc.vector.memset(x2a[:, PD + W:WP], 0.0)
            atmp2 = fp.tile([128, 512], F32, tag="atmp2", bufs=2)

            def epi2(ci, cs, cw, avp, rzb):
                nc.vector.tensor_mul(atmp2[:, 0:cw], avp[0:128, 0:cw],
                                     rzb[0:128, 0:cw])
                nc.vector.tensor_add(x2a[:, PD + cs:PD + cs + cw],
                                     atmp2[:, 0:cw], x2q[:, cs:cs + cw])
                if ci == 0:
                    nc.vector.memset(x2a[:, PD:PD + MG], 0.0)

            def mid2(cp_, zr_tile):
                if cp_[0] == 2:
                    nc.sync.dma_start(pw_in[1].ap(), zr_tile[0:64, 0:2])
                    nc.gpsimd.collective_compute(
                        "AllGather", ALU.bypass, replica_groups=PAIRS,
                        ins=[pw_in[1].ap().opt()],
                        outs=[pw_out[1].ap().opt()])

            attention(x2q, x2k, O_WK2, O_WQ2, O_WV2, 16, 128, True,
                      kq2, q2, vt2, epi2, mid_hook=mid2)

            # ---------------- conv3 (15 taps, tap-outer) + stats -----------
            c3banksA = sct()
            c3banksB = sct()
            c3small = avt()
            for t in range(15):
                for ci, (cs, cw) in enumerate(CHUNKS):
                    dst = _cbank((c3banksA, c3banksB), c3small, ci)[:, 0:cw]
                    nc.tensor.matmul(dst,
                                     cb[:, O_W3T + 128 * t:O_W3T + 128 * (t + 1)],
                                     x2a[:, PD + cs + t - 7:PD + cs + t - 7 + cw],
                                     start=(t == 0), stop=(t == 14))
            st3 = fp.tile([128, 12], F32, tag="st3")
            for ci, (cs, cw) in enumerate(CHUNKS):
                a, wd = STAT_SL[ci]
                sl = _cbank((c3banksA, c3banksB), c3small,
                            ci)[:, a - cs:a - cs + wd]
                nc.vector.tensor_reduce(st3[:, ci:ci + 1], sl, axis=AX.X,
                                        op=ALU.add)
                nc.scalar.activation(sq_scrA[:, 0:wd], sl, AF.Square,
                                     accum_out=st3[:, 5 + ci:6 + ci])
            stats3 = fp.tile([128, 2], F32, tag="stats3")
            nc.vector.tensor_reduce(stats3[:, 0:1], st3[:, 0:5], axis=AX.X,
                                    op=ALU.add)
            nc.vector.tensor_reduce(stats3[:, 1:2], st3[:, 5:10], axis=AX.X,
                                    op=ALU.add)
            nc.sync.dma_start(ar_in[1].ap(), stats3[:])
            nc.gpsimd.collective_compute(
                "AllGather", ALU.bypass, replica_groups=ALL8,
                ins=[ar_in[1].ap().opt()], outs=[ar_out[1].ap().opt()])
            dve_warm_chain(10, stats3[0:128, 0:2])

            ss3 = bn_from_ar(ar_out[1], C_BN3, "ss3")

            # x3d = relu(bn3(y3)) with dup+shift rows, straight from PSUM
            x3d = ap_.tile([128, WP], BF16, tag="x3d")
            nc.vector.memset(x3d[:, 0:PD], 0.0)
            nc.vector.memset(x3d[:, PD + W - 1:WP], 0.0)
            for ci, (cs, cw) in enumerate(CHUNKS):
                bank = _cbank((c3banksA, c3banksB), c3small, ci)
                src_lo = bank[0:64, 0:cw]
                src_hi = bank[64:128, 0:cw]
                nc.scalar.activation(x3d[0:64, PD + cs:PD + cs + cw], src_lo,
                                     AF.Relu, bias=ss3[0:64, 6:7],
                                     scale=ss3[0:64, 5:6])
                nc.vector.tensor_scalar(x3d[64:128, PD + cs - 1:PD + cs + cw - 1],
                                        src_hi, scalar1=ss3[64:128, 5:6],
                                        scalar2=ss3[64:128, 6:7],
                                        op0=ALU.mult, op1=ALU.add)
                nc.vector.tensor_scalar_max(
                    x3d[64:128, PD + cs - 1:PD + cs + cw - 1],
                    x3d[64:128, PD + cs - 1:PD + cs + cw - 1], 0.0)
            nc.vector.memset(x3d[0:64, PD:PD + MG], 0.0)
            nc.vector.memset(x3d[64:128, PD - 1:PD + MG - 1], 0.0)

            # ---------------- conv4 (tap pairs) + relu, dup+shift ----------
            x4q = ap_.tile([64, WP], BF16, tag="x4q")
            nc.vector.memset(x4q[:, 0:PD], 0.0)
            nc.vector.memset(x4q[:, PD + W - 1:WP], 0.0)
            c4banksA = sct()
            c4banksB = sct()
            c4small = avt()
            for t in range(8):
                for ci, (cs, cw) in enumerate(CHUNKS):
                    dst = _cbank((c4banksA, c4banksB), c4small, ci,
                                 rows=64)[:, 0:cw]
                    nc.tensor.matmul(dst,
                                     cb[:, O_W4P + 64 * t:O_W4P + 64 * (t + 1)],
                                     x3d[:, PD + cs + 2 * t - 7:PD + cs + 2 * t - 7 + cw],
                                     start=(t == 0), stop=(t == 7))
            for ci, (cs, cw) in enumerate(CHUNKS):
                bank4 = _cbank((c4banksA, c4banksB), c4small, ci, rows=64)
                lo = bank4[0:32, 0:cw]
                hi = bank4[32:64, 0:cw]
                nc.scalar.activation(x4q[0:32, PD + cs:PD + cs + cw], lo,
                                     AF.Relu, bias=cf[0:32, C_C4B:C_C4B + 1])
                nc.vector.tensor_scalar(x4q[32:64, PD + cs - 1:PD + cs + cw - 1],
                                        hi, scalar1=cf[32:64, C_C4B:C_C4B + 1],
                                        scalar2=0.0, op0=ALU.add, op1=ALU.max)
            nc.vector.memset(x4q[0:32, PD:PD + MG], 0.0)
            nc.vector.memset(x4q[32:64, PD - 1:PD + MG - 1], 0.0)

            # ---------------- conv5 (tap pairs) + output -------------------
            c5banksA = sct()
            c5banksB = sct()
            for t in range(8):
                for ci, (cs, cw) in enumerate(OUT_CHUNKS):
                    c5b_ = c5banksA if ci < 2 else c5banksB
                    nc.tensor.matmul(c5b_[0:1, ci % 2, 0:cw],
                                     cb[0:64, O_W5P + t:O_W5P + t + 1],
                                     x4q[:, PD + cs + 2 * t - 7:PD + cs + 2 * t - 7 + cw],
                                     start=(t == 0), stop=(t == 7))
            for ci, (cs, cw) in enumerate(OUT_CHUNKS):
                oc = fp.tile([1, 512], F32, tag="oc", bufs=2)
                c5b_ = c5banksA if ci < 2 else c5banksB
                nc.vector.tensor_scalar(oc[:, 0:cw], c5b_[0:1, ci % 2, 0:cw],
                                        scalar1=STRENGTH,
                                        scalar2=STRENGTH * c5b_f,
                                        op0=ALU.mult, op1=ALU.add)
                nc.vector.tensor_add(oc[:, 0:cw], oc[:, 0:cw],
                                     aud[:, cs - MG:cs - MG + cw])
                nc.sync.dma_start(out_d[:, cs - MG:cs - MG + cw], oc[:, 0:cw])

    nc.compile()
    return nc


def _host_prep(audio, message, w1, w2, w3, w4, w5, a1_wq, a1_wk, a1_wv, a1_g,
               a2_wq, a2_wk, a2_wv, a2_g, bn1_g, bn1_b, bn2_g, bn2_b,
               bn3_g, bn3_b, c4b):
    """Build per-core input dicts."""
    in_maps = []
    for core in range(NCORES):
        b, h = core // 2, core % 2
        rev = h == 1

        def fw(w):
            return w[:, :, ::-1] if rev else w

        w1f, w2f, w3f, w4f, w5f = (np.asarray(fw(x), np.float32)
                                   for x in (w1, w2, w3, w4, w5))

        # bf16 const blob
        cbm = np.zeros((128, NB), np.float32)
        # w1c_dup per batch group (group 0 = own orientation, others canonical)
        w1c_own = np.zeros((32, 128), np.float32)
        w1c_can = np.zeros((32, 128), np.float32)
        w1_can = np.asarray(w1, np.float32)
        for t in range(15):
            for ch in range(2):
                w1c_own[2 * t + ch, 0:64] = w1f[:, ch, t]
                w1c_own[2 * t + ch, 64:128] = w1f[:, ch, t]
                w1c_can[2 * t + ch, 0:64] = w1_can[:, ch, t]
                w1c_can[2 * t + ch, 64:128] = w1_can[:, ch, t]
        for g in range(4):
            cbm[32 * g:32 * g + 32, O_W1C:O_W1C + 128] = (w1c_own if g == 0
                                                          else w1c_can)
        # attn1 qk 4x col-replicated, row-duplicated
        wk1T = np.asarray(a1_wk, np.float32).T  # [64, 8]
        wq1T = np.asarray(a1_wq, np.float32).T
        for i in range(4):
            for u in range(2):
                cbm[64 * u:64 * u + 64, O_WK1 + 32 * i:O_WK1 + 32 * i + 8] = wk1T
                cbm[64 * u:64 * u + 64, O_WQ1 + 32 * i:O_WQ1 + 32 * i + 8] = wq1T
        wv1T = np.asarray(a1_wv, np.float32).T * float(np.asarray(a1_g))
        cbm[0:64, O_WV1:O_WV1 + 64] = wv1T
        cbm[64:128, O_WV1:O_WV1 + 64] = wv1T
        # attn2
        wk2T = np.asarray(a2_wk, np.float32).T  # [128, 16]
        wq2T = np.asarray(a2_wq, np.float32).T
        for i in range(4):
            cbm[:, O_WK2 + 32 * i:O_WK2 + 32 * i + 16] = wk2T
            cbm[:, O_WQ2 + 32 * i:O_WQ2 + 32 * i + 16] = wq2T
        cbm[:, O_WV2:O_WV2 + 128] = np.asarray(a2_wv, np.float32).T * float(
            np.asarray(a2_g))
        # conv2 tap pairs [128ch_in x 2 taps, 128 out]
        for t in range(8):
            blk = np.zeros((128, 128), np.float32)
            blk[0:64, :] = w2f[:, :, 2 * t].T
            if 2 * t + 1 < 15:
                blk[64:128, :] = w2f[:, :, 2 * t + 1].T
            cbm[:, O_W2P + 128 * t:O_W2P + 128 * (t + 1)] = blk
        # conv3 taps, out-duplicated
        for t in range(15):
            blk = np.zeros((128, 128), np.float32)
            blk[:, 0:64] = w3f[:, :, t].T
            blk[:, 64:128] = w3f[:, :, t].T
            cbm[:, O_W3T + 128 * t:O_W3T + 128 * (t + 1)] = blk
        # conv4 tap pairs, out-duplicated [64 out]
        for t in range(8):
            blk = np.zeros((128, 64), np.float32)
            blk[0:64, 0:32] = w4f[:, :, 2 * t].T
            blk[0:64, 32:64] = w4f[:, :, 2 * t].T
            if 2 * t + 1 < 15:
                blk[64:128, 0:32] = w4f[:, :, 2 * t + 1].T
                blk[64:128, 32:64] = w4f[:, :, 2 * t + 1].T
            cbm[:, O_W4P + 64 * t:O_W4P + 64 * (t + 1)] = blk
        # conv5 tap pairs [32ch x 2 shifts, 1]
        for t in range(8):
            cbm[0:32, O_W5P + t] = w5f[0, :, 2 * t]
            if 2 * t + 1 < 15:
                cbm[32:64, O_W5P + t] = w5f[0, :, 2 * t + 1]

        # f32 const blob
        cfm = np.zeros((128, NF), np.float32)
        for col, g_, b_ in ((C_BN1, bn1_g, bn1_b), (C_BN3, bn3_g, bn3_b)):
            gg = np.asarray(g_, np.float32); bb = np.asarray(b_, np.float32)
            cfm[0:64, col] = gg; cfm[64:128, col] = gg
            cfm[0:64, col + 1] = bb; cfm[64:128, col + 1] = bb
        cfm[:, C_BN2] = np.asarray(bn2_g, np.float32)
        cfm[:, C_BN2 + 1] = np.asarray(bn2_b, np.float32)
        c4 = np.asarray(c4b, np.float32)
        cfm[0:32, C_C4B] = c4; cfm[32:64, C_C4B] = c4

        # x0 im2col blob: batch group g at partitions 32g (+2t+ch), left-pad 32
        x0m = np.zeros((128, XW), np.float32)
        order = [b] + [x for x in range(B) if x != b]
        for g, bb in enumerate(order):
            arow = np.asarray(audio[bb, 0], np.float32)
            mrow = np.asarray(message[bb, 0], np.float32)
            if g == 0 and rev:
                arow = arow[::-1]; mrow = mrow[::-1]
            # group 0 rows pair with w1c_own (tap-flipped for rev cores),
            # groups 1-3 with w1c_can - handled in the cb blob
            xp = np.zeros((2, L + 96), np.float32)  # pos -48..L+48
            xp[0, 48:48 + L] = arow
            xp[1, 48:48 + L] = mrow
            for t in range(15):
                for ch in range(2):
                    # col c -> pos c-32; im2col val = x[ch, pos + t - 7]
                    # pos+t-7 for c in [0, 4128): range [-39+t, 4089+t)
                    lo = -32 + t - 7
                    x0m[32 * g + 2 * t + ch, 0:4128] = xp[ch, 48 + lo:48 + lo + 4128]
        aud_own = np.asarray(audio[b, 0], np.float32)
        if rev:
            aud_own = aud_own[::-1]
        im = {
            "cb": _bf(cbm),
            "cf": _f32(cfm),
            "x0": _bf(x0m),
            "aud": _f32(aud_own[0:OWN].reshape(1, OWN)),
        }
        in_maps.append(im)
    return in_maps


def kernel(audio, message, w1, c1b, w2, c2b, w3, c3b, w4, c4b, w5, c5b,
           bn1_g, bn1_b, bn2_g, bn2_b, bn3_g, bn3_b,
           a1_wq, a1_wk, a1_wv, a1_g, a2_wq, a2_wk, a2_wv, a2_g,
           _trace=False):
    global LAST_RESULTS
    audio = np.asarray(audio); message = np.asarray(message)
    # c1b/c2b/c3b cancel inside training-mode BatchNorm; c4b/c5b applied.
    in_maps = _host_prep(audio, message, w1, w2, w3, w4, w5,
                         a1_wq, a1_wk, a1_wv, a1_g, a2_wq, a2_wk, a2_wv, a2_g,
                         bn1_g, bn1_b, bn2_g, bn2_b, bn3_g, bn3_b, c4b)
    nc = build_graph(float(np.asarray(c5b).reshape(-1)[0]))
    res = run_bass_kernel_spmd(nc, in_maps, core_ids=list(range(NCORES)),
                               trace=_trace)
    LAST_RESULTS = res

    out = np.zeros((B, 1, L), np.float32)
    for core in range(NCORES):
        b, h = core // 2, core % 2
        o = res.results[core]["out"][0]
        if h == 0:
            out[b, 0, 0:OWN] = o
        else:
            out[b, 0, OWN:L] = o[::-1]
    return out


# revision 25
# speedup vs baseline: 1.0294x; 1.0294x over previous
"""Trainium2 Bass kernel for AdvancedAudioStegEncoder (B=4, L=4096, 8 cores).

Sharding: hybrid batch x sequence. Core c handles batch b=c//2, half h=c%2.
h=1 cores receive x-REVERSED data + tap-flipped conv kernels from the host,
so every core's own half is the LEFT half of its oriented sequence and all
graph slice offsets are uniform (required for SPMD single-graph execution).

Key structure (vs the straightforward port):
  - conv1 is computed over ALL 4 batches' full length on every core (cheap:
    4-way row-packed matmuls, one per batch concurrently in the PE array).
    BN1 batch stats are then computed locally -> NO collective for BN1, and
    attn1 keys/values for the full 4096 positions are local -> NO AllGather.
    The BN1 mean uses sum(y1) = w1c^T @ colsum(x0) (one tiny matmul).
  - BN2: AllReduce of stats runs CONCURRENTLY with a pair-AllGather of the
    pre-BN conv2 output (y2); remote K/V for attn2 are computed locally
    after BN2 arrives. 2 collective stall windows total (BN2, BN3).
  - Attention scores matmuls (contraction d=8/16) are 4-way row-packed via
    tile_position with 4x-replicated projection weights; exp runs 2048 wide
    across the 4 score PSUM banks in one ScalarE instruction.
  - attn softmax normalizer: attn1 via a 65th ones-column of V^T; attn2 via
    ones-lhsT matmuls over pair-pre-reduced probs (DVE adds halve the
    extra PE streams). gamma is folded into the V projection weights.
  - BN rstd via exp(-0.5*ln(var)) so only one ACT table set (exp+ln) is
    ever loaded; BN apply is fused scale+bias+relu on ScalarE/VectorE.
  - Dummy matmul->copy chains keep the PE HAM clock warm through the two
    collective stalls; a dummy AllGather at kernel start absorbs core skew.
"""
import sys
import numpy as np

sys.path.insert(0, "/opt/trn_rl_repo")

import ml_dtypes

import concourse.bass as bass
import concourse.bacc as bacc
import concourse.tile as tile
import concourse.mybir as mybir
from concourse.bass_utils import run_bass_kernel_spmd

BF16 = mybir.dt.bfloat16
F32 = mybir.dt.float32
AF = mybir.ActivationFunctionType
ALU = mybir.AluOpType
AX = mybir.AxisListType

B, L = 4, 4096
NCORES = 8
OWN = 2048
MG = 32
W = OWN + 2 * MG          # 2112 query window (cols 0..2112 = pos -32..2080)
XW = 4160                 # x-full cols: [0,4128) = pos [-32,4096), + 32 pad
KEY0 = MG                 # keys = cols [32, 4128) = pos [0, 4096)
NK = 4096
PD = 8                    # zero pad around conv input tiles
WP = W + 2 * PD           # 2128
EPS = 1e-5
NSTAT = float(B * L)
STRENGTH = 0.01
JT = 32                   # key tiles of 128

# query-window chunks
CHUNKS = [(0, 512), (512, 512), (1024, 512), (1536, 512), (2048, 64)]
CPAIRS = [(0, 1), (2, 3), (4,)]
# own-real stat slices within the window
STAT_SL = [(32, 480), (512, 512), (1024, 512), (1536, 512), (2048, 32)]
# x-full chunks (9 per batch, conv1 / x1full apply)
FCHUNKS = [(i * 512, 512) for i in range(8)] + [(4096, 32)]
# stat slices for conv1 within x-full chunks (exclude cols [0,32) = pos<0)
FSTAT_SL = [(32, 480)] + [(i * 512, 512) for i in range(1, 8)] + [(4096, 32)]
OUT_CHUNKS = [(32, 512), (544, 512), (1056, 512), (1568, 512)]

LAST_RESULTS = None


def _bf(x):
    return np.ascontiguousarray(x).astype(ml_dtypes.bfloat16)


def _f32(x):
    return np.ascontiguousarray(x).astype(np.float32)


# ---------------- constant blob layout (bf16, [128, NB]) ----------------
# offsets (cols)
O_W1C = 0          # [128,128] w1c_dup per batch row-group
O_WK1 = 128        # [128,128] wk1 4x col-replicated, row-duplicated
O_WQ1 = 256
O_WV1 = 384        # [128,64] wv1^T * g1, row-duplicated
O_WK2 = 448        # [128,128] wk2 4x col-replicated (16 used + 16 zero each)
O_WQ2 = 576
O_WV2 = 704        # [128,128] wv2^T * g2
O_W2P = 832        # [128, 8*128] conv2 tap pairs
O_W3T = 1856       # [128, 15*128] conv3 taps (out-duplicated)
O_W4P = 3776       # [128, 8*64] conv4 tap pairs (out-duplicated)
O_W5P = 4288       # [64, 8] conv5 tap pairs
NB = 4296

# constf blob (f32, [128, NF])
C_BN1 = 0   # g,b dup  (cols 0,1)
C_BN2 = 2
C_BN3 = 4   # dup
C_C4B = 6   # c4b dup (rows 0:64)
NF = 8


def build_graph(c5b_f: float):
    nc = bacc.Bacc("TRN2", target_bir_lowering=False, debug=False,
                   num_devices=NCORES)

    def din(name, shape, dt=BF16):
        return nc.dram_tensor(name, shape, dt, kind="ExternalInput")

    cb_d = din("cb", [128, NB])
    cf_d = din("cf", [128, NF], F32)
    x0_d = din("x0", [128, XW])
    aud_d = din("aud", [1, OWN], F32)

    out_d = nc.dram_tensor("out", [1, OWN], F32, kind="ExternalOutput")

    # collective buffers
    dum_in = nc.dram_tensor("dum_in", [128, 2], F32, kind="Internal")
    dum_out = nc.dram_tensor("dum_out", [8, 128, 2], F32, kind="Internal",
                             addr_space="Shared")
    ar_in = [nc.dram_tensor(f"ar{i}_in", [128, 2], F32, kind="Internal")
             for i in range(2)]
    pw_in = [nc.dram_tensor(f"pw{i}_in", [64, 2], F32, kind="Internal")
             for i in range(2)]
    pw_out = [nc.dram_tensor(f"pw{i}_out", [8, 64, 2], F32, kind="Internal",
                             addr_space="Shared") for i in range(2)]
    ar_out = [nc.dram_tensor(f"ar{i}_out", [8, 128, 2], F32, kind="Internal",
                             addr_space="Shared") for i in range(2)]
    AGN = 128 * OWN
    ag_in = nc.dram_tensor("ag_in", [AGN], BF16, kind="Internal")
    ag_out = nc.dram_tensor("ag_out", [2, AGN], BF16, kind="Internal")

    PAIRS = [[0, 1], [2, 3], [4, 5], [6, 7]]
    ALL8 = [list(range(8))]

    with tile.TileContext(nc) as tc:
        with tc.tile_pool(name="const", bufs=1) as cp, \
             tc.tile_pool(name="act", bufs=1) as ap_, \
             tc.tile_pool(name="flow", bufs=1) as fp, \
             tc.tile_pool(name="ps", bufs=1, space="PSUM") as ps:

            # ---------------- loads ----------------
            cf = cp.tile([128, NF], F32, tag="cf")
            nc.sync.dma_start(cf[:], cf_d.ap())
            x0 = cp.tile([128, XW], BF16, tag="x0")
            nc.sync.dma_start(x0[:], x0_d.ap())
            cb = cp.tile([128, NB], BF16, tag="cb")
            nc.scalar.dma_start(cb[:], cb_d.ap())
            aud = cp.tile([1, OWN], F32, tag="aud")
            nc.sync.dma_start(aud[:], aud_d.ap())

            ones128 = cp.tile([128, 1], BF16, tag="ones128")
            nc.vector.memset(ones128[:], 1.0)

            # dummy collective to absorb initial core skew + warm CC path
            nc.sync.dma_start(dum_in.ap(), cf[:, 0:2])
            nc.gpsimd.collective_compute(
                "AllGather", ALU.bypass, replica_groups=ALL8,
                ins=[dum_in.ap().opt()], outs=[dum_out.ap().opt()])

            # psum tag helpers -------------------------------------------------
            _nid = [0]

            def sct():
                _nid[0] += 1
                return ps.tile([128, 2, 512], F32, tag="sc", bufs=2,
                               name=f"sc{_nid[0]}")

            def avt():
                _nid[0] += 1
                return ps.tile([128, 512], F32, tag="av", bufs=2,
                               name=f"av{_nid[0]}")

            def zzt():
                _nid[0] += 1
                return ps.tile([128, 512], F32, tag="zz", bufs=2,
                               name=f"zz{_nid[0]}")

            # ---------------- conv1: all 4 batches, 4-way row-packed --------
            y1own = ap_.tile([128, XW], BF16, tag="y1own")
            sq_scrA = fp.tile([128, 512], F32, tag="sq_scrA")  # ACT scratch
            sq_scrD = fp.tile([128, 512], F32, tag="sq_scrD")  # DVE scratch
            sq_scrD2 = fp.tile([128, 512], F32, tag="sq_scrD2")
            st1 = fp.tile([128, 40], F32, tag="st1")  # sumsq slots
            nslot = [0]
            for ci, (cs, cw) in enumerate(FCHUNKS):
                # alternate bank sets so stats of span n overlap span n+1
                if ci % 2 == 0:
                    spanA = sct()
                    spanB = sct()
                    banks = [spanA[:, 0, :], spanA[:, 1, :],
                             spanB[:, 0, :], spanB[:, 1, :]]
                else:
                    quad = [ps.tile([128, 512], F32, tag="av", bufs=2, name=f"cq{ci}a"),
                            ps.tile([128, 512], F32, tag="av", bufs=2, name=f"cq{ci}b"),
                            ps.tile([128, 512], F32, tag="zz", bufs=2, name=f"cq{ci}c"),
                            ps.tile([128, 512], F32, tag="zz", bufs=2, name=f"cq{ci}d")]
                    banks = [q_[0:128, :] for q_ in quad]
                for b in range(4):
                    nc.tensor.matmul(
                        banks[b][:, 0:cw],
                        cb[32 * b:32 * b + 30, O_W1C:O_W1C + 128],
                        x0[32 * b:32 * b + 30, cs:cs + cw],
                        start=True, stop=True, tile_position=(32 * b, 0))
                a, wd = FSTAT_SL[ci]
                o = a - cs
                for b in range(4):
                    sl = banks[b][:, o:o + wd]
                    slot = nslot[0]; nslot[0] += 1
                    if b != 1:
                        # ACT: square+accum straight from PSUM
                        nc.scalar.activation(sq_scrA[:, 0:wd], sl, AF.Square,
                                             accum_out=st1[:, slot:slot + 1])
                    else:
                        # DVE: copy out, square, reduce (DVE can't read two
                        # PSUM operands)
                        nc.vector.tensor_copy(sq_scrD[:, 0:wd], sl)
                        nc.vector.tensor_mul(sq_scrD2[:, 0:wd],
                                             sq_scrD[:, 0:wd],
                                             sq_scrD[:, 0:wd])
                        nc.vector.tensor_reduce(st1[:, slot:slot + 1],
                                                sq_scrD2[:, 0:wd],
                                                axis=AX.X, op=ALU.add)
                # own batch (group 0): copy out y1 (DVE; ACT is square-bound)
                nc.vector.tensor_copy(y1own[:, cs:cs + cw],
                                      banks[0][:, 0:cw])

            # mean via colsum trick: s0 = rowsum(x0 over stat cols)
            s0 = fp.tile([128, 2], F32, tag="s0")
            s0b = fp.tile([128, 1], BF16, tag="s0b")
            nc.vector.tensor_reduce(s0[:, 0:1], x0[:, 32:4128],
                                    axis=AX.X, op=ALU.add)
            nc.vector.tensor_copy(s0b[:], s0[:, 0:1])
            sum1ps = avt()
            nc.tensor.matmul(sum1ps[0:128, 0:1], cb[:, O_W1C:O_W1C + 128],
                             s0b[:], start=True, stop=True)
            # total sumsq
            ssq1 = fp.tile([128, 2], F32, tag="ssq1")
            nc.vector.tensor_reduce(ssq1[:, 0:1], st1[:, 0:36],
                                    axis=AX.X, op=ALU.add)

            def bn_scale_shift(sum_ap, sumsq_ap, gb_col, tag):
                """sum/sumsq [128,1] -> (scale, shift) [128,1] f32 via ln/exp"""
                s = fp.tile([128, 8], F32, tag=tag)
                nc.vector.tensor_scalar_mul(s[:, 0:1], sum_ap, 1.0 / NSTAT)
                nc.vector.tensor_scalar_mul(s[:, 1:2], sumsq_ap, 1.0 / NSTAT)
                nc.vector.tensor_mul(s[:, 2:3], s[:, 0:1], s[:, 0:1])
                nc.vector.tensor_sub(s[:, 2:3], s[:, 1:2], s[:, 2:3])
                nc.vector.tensor_scalar_add(s[:, 2:3], s[:, 2:3], EPS)
                # rstd = exp(-0.5*ln(var))
                nc.scalar.activation(s[:, 3:4], s[:, 2:3], AF.Ln)
                nc.scalar.activation(s[:, 4:5], s[:, 3:4], AF.Exp, scale=-0.5)
                nc.vector.tensor_mul(s[:, 5:6], s[:, 4:5],
                                     cf[:, gb_col:gb_col + 1])
                nc.vector.tensor_mul(s[:, 6:7], s[:, 0:1], s[:, 5:6])
                nc.vector.tensor_sub(s[:, 6:7], cf[:, gb_col + 1:gb_col + 2],
                                     s[:, 6:7])
                return s  # scale = s[:,5:6], shift = s[:,6:7]

            ss1 = bn_scale_shift(sum1ps[0:128, 0:1], ssq1[:, 0:1], C_BN1, "ss1")

            # ---------------- x1full = relu(bn1(y1own)), dup rows ----------
            x1f = ap_.tile([128, XW], BF16, tag="x1f")
            for ci, (cs, cw) in enumerate(FCHUNKS):
                if ci % 2 == 0:
                    nc.scalar.activation(x1f[:, cs:cs + cw], y1own[:, cs:cs + cw],
                                         AF.Relu, bias=ss1[:, 6:7],
                                         scale=ss1[:, 5:6])
                else:
                    nc.vector.tensor_scalar(x1f[:, cs:cs + cw],
                                            y1own[:, cs:cs + cw],
                                            scalar1=ss1[:, 5:6],
                                            scalar2=ss1[:, 6:7],
                                            op0=ALU.mult, op1=ALU.add)
                    nc.vector.tensor_scalar_max(x1f[:, cs:cs + cw],
                                                x1f[:, cs:cs + cw], 0.0)
            nc.vector.memset(x1f[:, 0:MG], 0.0)  # OOB query margin

            wch = [fp.tile([128, 512], BF16, tag=f"wch{i}", name=f"wch{i}")
                   for i in range(2)]
            nc.vector.memset(wch[0][:], 0.001)
            nc.vector.memset(wch[1][:], 0.001)

            def dve_warm_chain(n, gate_f32_ap):
                """Keep the PE HAM clock warm through a collective stall:
                matmul every ~1.2us, paced by DVE psum->sbuf copies."""
                nc.vector.tensor_scalar_mul(wch[0][:, 0:2], gate_f32_ap, 1e-3)
                for k in range(n):
                    t = zzt()
                    nc.tensor.matmul(t[0:128, 0:512], cb[0:128, 0:128],
                                     wch[k % 2][:], start=True, stop=True)
                    nc.vector.tensor_scalar_mul(wch[(k + 1) % 2][:],
                                                t[:, 0:512], 1e-3)

            # ---------------- attention (shared for both layers) -----------
            def attention(xq, xk, wk_off, wq_off, wv_off, d, vcols, use_zz,
                          kq_sb, q_sb, vt_sb, epilogue, mid_hook=None):
                """xq [128, W] (dup rows for attn1), xk [128, NK] keys source.
                kq/q 4x-replicated projections; vt built per 128-key tile.
                epilogue(ci, cs, cw, av_ps, rzb) writes the output chunk."""
                # k projection over keys (8 chunks of 512)
                for c2 in range(0, 8, 2):
                    sp = sct()
                    if d == 8:  # 2-way packed (K=64, dup rows)
                        for u in range(2):
                            nc.tensor.matmul(
                                sp[:, u, :],
                                cb[64 * u:64 * u + 64, wk_off:wk_off + 128],
                                xk[64 * u:64 * u + 64,
                                   512 * (c2 + u):512 * (c2 + u) + 512],
                                start=True, stop=True,
                                tile_position=(64 * u, 0))
                    else:
                        for u in range(2):
                            nc.tensor.matmul(
                                sp[:, u, :], cb[:, wk_off:wk_off + 128],
                                xk[:, 512 * (c2 + u):512 * (c2 + u) + 512],
                                start=True, stop=True)
                    if (c2 // 2) % 2 == 0:
                        nc.scalar.activation(kq_sb[:, 512 * c2:512 * (c2 + 2)],
                                             sp[:, 0:2, :], AF.Copy)
                    else:
                        nc.vector.tensor_copy(kq_sb[:, 512 * c2:512 * (c2 + 2)],
                                              sp[:, 0:2, :])
                # q projection over window (5 chunks)
                spq1 = sct()
                spq2 = sct()
                for ci in range(4):
                    spq = spq1 if ci < 2 else spq2
                    if d == 8:
                        u = ci % 2
                        nc.tensor.matmul(
                            spq[:, ci % 2, :],
                            cb[64 * u:64 * u + 64, wq_off:wq_off + 128],
                            xq[64 * u:64 * u + 64, 512 * ci:512 * ci + 512],
                            start=True, stop=True, tile_position=(64 * u, 0))
                    else:
                        nc.tensor.matmul(
                            spq[:, ci % 2, :], cb[:, wq_off:wq_off + 128],
                            xq[:, 512 * ci:512 * ci + 512],
                            start=True, stop=True)
                nc.scalar.activation(q_sb[:, 0:1024], spq1[:, 0:2, :],
                                     AF.Copy)
                nc.vector.tensor_copy(q_sb[:, 1024:2048], spq2[:, 0:2, :])
                q5 = avt()
                if d == 8:
                    nc.tensor.matmul(q5[0:128, 0:64],
                                     cb[0:64, wq_off:wq_off + 128],
                                     xq[0:64, 2048:2112],
                                     start=True, stop=True)
                else:
                    nc.tensor.matmul(q5[0:128, 0:64], cb[:, wq_off:wq_off + 128],
                                     xq[:, 2048:2112], start=True, stop=True)
                nc.vector.tensor_copy(q_sb[:, 2048:2112], q5[:, 0:64])

                # vT per key tile
                for j2 in range(0, JT, 2):
                    va = avt()
                    vb = avt()
                    if d == 8:
                        nc.tensor.matmul(va[0:128, 0:64],
                                         xk[0:64, 128 * j2:128 * j2 + 128],
                                         cb[0:64, wv_off:wv_off + 64],
                                         start=True, stop=True)
                        nc.tensor.matmul(vb[0:128, 0:64],
                                         xk[64:128, 128 * (j2 + 1):128 * (j2 + 1) + 128],
                                         cb[64:128, wv_off:wv_off + 64],
                                         start=True, stop=True,
                                         tile_position=(64, 0))
                        nc.vector.tensor_copy(vt_sb[:, j2, 0:64], va[:, 0:64])
                        nc.vector.tensor_copy(vt_sb[:, j2 + 1, 0:64], vb[:, 0:64])
                    else:
                        nc.tensor.matmul(va[0:128, 0:128],
                                         xk[:, 128 * j2:128 * j2 + 128],
                                         cb[:, wv_off:wv_off + 128],
                                         start=True, stop=True)
                        nc.tensor.matmul(vb[0:128, 0:128],
                                         xk[:, 128 * (j2 + 1):128 * (j2 + 1) + 128],
                                         cb[:, wv_off:wv_off + 128],
                                         start=True, stop=True)
                        if (j2 // 2) % 2 == 0:
                            nc.scalar.activation(vt_sb[:, j2, 0:128],
                                                 va[:, 0:128], AF.Copy)
                            nc.scalar.activation(vt_sb[:, j2 + 1, 0:128],
                                                 vb[:, 0:128], AF.Copy)
                        else:
                            nc.vector.tensor_copy(vt_sb[:, j2, 0:128],
                                                  va[:, 0:128])
                            nc.vector.tensor_copy(vt_sb[:, j2 + 1, 0:128],
                                                  vb[:, 0:128])
                if vcols == 65:
                    nc.vector.memset(vt_sb[:, :, 64:65], 1.0)

                # main loop over chunk pairs
                for cp_ in CPAIRS:
                    avps = {ci: avt() for ci in cp_}
                    zzps = {ci: zzt() for ci in cp_} if use_zz else {}
                    for jg in range(8):
                        prb = {}
                        for ci in cp_:
                            cs, cw = CHUNKS[ci]
                            p = fp.tile([128, 4, 512], BF16, tag="probs",
                                        bufs=4, name=f"p{ci}")
                            for half in range(2):
                                sp = sct()
                                for u in range(2):
                                    i = 2 * half + u
                                    j = 4 * jg + i
                                    nc.tensor.matmul(
                                        sp[:, u, 0:cw],
                                        kq_sb[32 * i:32 * i + d,
                                              128 * j:128 * j + 128],
                                        q_sb[32 * i:32 * i + d, cs:cs + cw],
                                        start=True, stop=True,
                                        tile_position=(32 * i, 0))
                                nc.scalar.activation(
                                    p[:, 2 * half:2 * half + 2, 0:cw],
                                    sp[:, :, 0:cw], AF.Exp)
                            prb[ci] = p
                        # av: i-outer so each vT tile is loaded once per pair
                        for i in range(4):
                            j = 4 * jg + i
                            for ci in cp_:
                                cs, cw = CHUNKS[ci]
                                nc.tensor.matmul(
                                    avps[ci][0:vcols, 0:cw], vt_sb[:, j, :],
                                    prb[ci][:, i, 0:cw],
                                    start=(jg == 0 and i == 0),
                                    stop=(jg == 7 and i == 3))
                        if use_zz:
                            for ci in cp_:
                                cs, cw = CHUNKS[ci]
                                p = prb[ci]
                                p01 = fp.tile([128, 512], BF16, tag="p01",
                                              bufs=4, name=f"p01_{ci}")
                                p23 = fp.tile([128, 512], BF16, tag="p23",
                                              bufs=4, name=f"p23_{ci}")
                                nc.vector.tensor_add(p01[:, 0:cw], p[:, 0, 0:cw],
                                                     p[:, 1, 0:cw])
                                nc.vector.tensor_add(p23[:, 0:cw], p[:, 2, 0:cw],
                                                     p[:, 3, 0:cw])
                                nc.tensor.matmul(zzps[ci][0:1, 0:cw], ones128[:],
                                                 p01[:, 0:cw],
                                                 start=(jg == 0), stop=False)
                                nc.tensor.matmul(zzps[ci][0:1, 0:cw], ones128[:],
                                                 p23[:, 0:cw],
                                                 start=False, stop=(jg == 7))
                    # epilogue for this chunk pair: one batched reciprocal
                    # (second Z row parked at partition 32 for AP alignment)
                    zr = fp.tile([64, 512], F32, tag="zr", bufs=2, name="zr")
                    nc.vector.memset(zr[:], 1.0)
                    for k, ci in enumerate(cp_):
                        cs, cw = CHUNKS[ci]
                        zrow = (zzps[ci][0:1, 0:cw] if use_zz
                                else avps[ci][64:65, 0:cw])
                        nc.vector.tensor_copy(zr[32 * k:32 * k + 1, 0:cw],
                                              zrow)
                    nc.vector.reciprocal(zr[0:64, :], zr[0:64, :])
                    for k, ci in enumerate(cp_):
                        cs, cw = CHUNKS[ci]
                        if k == 0:
                            zsrc = zr[0:1, 0:cw]
                        else:
                            zc = fp.tile([1, 512], F32, tag="zc", bufs=2,
                                         name="zc")
                            nc.vector.tensor_copy(zc[:, 0:cw],
                                                  zr[32:33, 0:cw])
                            zsrc = zc[:, 0:cw]
                        rzb_ = fp.tile([128, 512], F32, tag="rzb", bufs=2,
                                       name=f"rzb{ci}")
                        nc.gpsimd.partition_broadcast(rzb_[:, 0:cw], zsrc)
                        epilogue(ci, cs, cw, avps[ci], rzb_)
                    if mid_hook is not None:
                        mid_hook(cp_, zr)

            # ---------------- attn1 ----------------
            kq1 = ap_.tile([128, NK], BF16, tag="kq1")
            q1 = ap_.tile([128, W], BF16, tag="q1")
            vt1 = ap_.tile([128, JT, 65], BF16, tag="vt1")
            x1ad = ap_.tile([128, WP], BF16, tag="x1ad")
            nc.vector.memset(x1ad[:, 0:PD], 0.0)
            nc.vector.memset(x1ad[:, PD + W - 1:WP], 0.0)
            atmp = fp.tile([64, 512], F32, tag="atmp", bufs=2)

            def epi1(ci, cs, cw, avp, rzb):
                nc.vector.tensor_mul(atmp[:, 0:cw], avp[0:64, 0:cw],
                                     rzb[0:64, 0:cw])
                nc.vector.tensor_add(x1ad[0:64, PD + cs:PD + cs + cw],
                                     atmp[:, 0:cw], x1f[0:64, cs:cs + cw])
                nc.vector.tensor_add(x1ad[64:128, PD + cs - 1:PD + cs + cw - 1],
                                     atmp[:, 0:cw], x1f[0:64, cs:cs + cw])
                if ci == 0:
                    nc.vector.memset(x1ad[0:64, PD:PD + MG], 0.0)
                    nc.vector.memset(x1ad[64:128, PD - 1:PD + MG - 1], 0.0)

            def mid1(cp_, zr_tile):
                if cp_[0] == 2:  # after chunk pair (2,3): ~2/3 through attn1
                    nc.sync.dma_start(pw_in[0].ap(), zr_tile[0:64, 0:2])
                    nc.gpsimd.collective_compute(
                        "AllGather", ALU.bypass, replica_groups=ALL8,
                        ins=[pw_in[0].ap().opt()],
                        outs=[pw_out[0].ap().opt()])

            attention(x1f[:, 0:W], x1f[:, KEY0:KEY0 + NK], O_WK1, O_WQ1, O_WV1,
                      8, 65, False, kq1, q1, vt1, epi1, mid_hook=mid1)

            # ---------------- conv2 (tap pairs, tap-outer) + stats ---------
            y2 = ap_.tile([128, W], BF16, tag="y2")
            c2banksA = sct()
            c2banksB = sct()
            c2small = avt()

            def _cbank(tiles, small, ci, rows=128):
                ta, tb = tiles
                if ci < 2:
                    return ta[0:rows, ci, :]
                if ci < 4:
                    return tb[0:rows, ci - 2, :]
                return small[0:rows, :]
            for t in range(8):
                for ci, (cs, cw) in enumerate(CHUNKS):
                    dst = _cbank((c2banksA, c2banksB), c2small, ci)[:, 0:cw]
                    nc.tensor.matmul(dst, cb[:, O_W2P + 128 * t:O_W2P + 128 * (t + 1)],
                                     x1ad[:, PD + cs + 2 * t - 7:PD + cs + 2 * t - 7 + cw],
                                     start=(t == 0), stop=(t == 7))
            st2 = fp.tile([128, 12], F32, tag="st2")
            for ci, (cs, cw) in enumerate(CHUNKS):
                bank = _cbank((c2banksA, c2banksB), c2small, ci)
                src = bank[:, 0:cw]
                a, wd = STAT_SL[ci]
                sl = bank[:, a - cs:a - cs + wd]
                nc.vector.tensor_reduce(st2[:, ci:ci + 1], sl, axis=AX.X,
                                        op=ALU.add)
                nc.scalar.activation(sq_scrA[:, 0:wd], sl, AF.Square,
                                     accum_out=st2[:, 5 + ci:6 + ci])
                if ci % 2 == 0:
                    nc.scalar.activation(y2[:, cs:cs + cw], src, AF.Copy)
                else:
                    nc.vector.tensor_copy(y2[:, cs:cs + cw], src)
            stats2 = fp.tile([128, 2], F32, tag="stats2")
            nc.vector.tensor_reduce(stats2[:, 0:1], st2[:, 0:5], axis=AX.X,
                                    op=ALU.add)
            nc.vector.tensor_reduce(stats2[:, 1:2], st2[:, 5:10], axis=AX.X,
                                    op=ALU.add)
            nc.sync.dma_start(ar_in[0].ap(), stats2[:])
            nc.gpsimd.collective_compute(
                "AllGather", ALU.bypass, replica_groups=ALL8,
                ins=[ar_in[0].ap().opt()], outs=[ar_out[0].ap().opt()])
            # concurrent pair-AllGather of pre-BN y2 (own real region)
            nc.sync.dma_start(
                ag_in.ap().rearrange("(p c) -> p c", p=128),
                y2[:, MG:MG + OWN])
            nc.gpsimd.collective_compute(
                "AllGather", ALU.bypass, replica_groups=PAIRS,
                ins=[ag_in.ap().opt()], outs=[ag_out.ap().opt()])
            dve_warm_chain(10, stats2[0:128, 0:2])

            def bn_from_ar(ar_dram, gb_col, tag):
                s8 = fp.tile([128, 8, 2], F32, tag=tag + "g")
                nc.sync.dma_start(
                    s8[:], ar_dram.ap().rearrange("b p c -> p b c"))
                sred = fp.tile([128, 2], F32, tag=tag + "r")
                nc.vector.tensor_reduce(sred[:, 0:2],
                                        s8[:].rearrange("p b c -> p c b"),
                                        axis=AX.X, op=ALU.add)
                return bn_scale_shift(sred[:, 0:1], sred[:, 1:2], gb_col, tag)

            ss2 = bn_from_ar(ar_out[0], C_BN2, "ss2")

            # x2q (own window) and x2k (gathered pair keys)
            x2q = ap_.tile([128, W], BF16, tag="x2q")
            for ci, (cs, cw) in enumerate(CHUNKS):
                if ci % 2 == 0:
                    nc.scalar.activation(x2q[:, cs:cs + cw], y2[:, cs:cs + cw],
                                         AF.Relu, bias=ss2[:, 6:7],
                                         scale=ss2[:, 5:6])
                else:
                    nc.vector.tensor_scalar(x2q[:, cs:cs + cw],
                                            y2[:, cs:cs + cw],
                                            scalar1=ss2[:, 5:6],
                                            scalar2=ss2[:, 6:7],
                                            op0=ALU.mult, op1=ALU.add)
                    nc.vector.tensor_scalar_max(x2q[:, cs:cs + cw],
                                                x2q[:, cs:cs + cw], 0.0)
            nc.vector.memset(x2q[:, 0:MG], 0.0)
            x2kr = ap_.tile([128, NK], BF16, tag="x2kr")
            for blk in range(2):
                nc.sync.dma_start(
                    x2kr[:, OWN * blk:OWN * (blk + 1)],
                    ag_out[blk].rearrange("(p c) -> p c", p=128))
            x2k = ap_.tile([128, NK], BF16, tag="x2k")
            for c8 in range(8):
                sl = slice(512 * c8, 512 * (c8 + 1))
                if c8 % 2 == 0:
                    nc.scalar.activation(x2k[:, sl], x2kr[:, sl], AF.Relu,
                                         bias=ss2[:, 6:7], scale=ss2[:, 5:6])
                else:
                    nc.vector.tensor_scalar(x2k[:, sl], x2kr[:, sl],
                                            scalar1=ss2[:, 5:6],
                                            scalar2=ss2[:, 6:7],
                                            op0=ALU.mult, op1=ALU.add)
                    nc.vector.tensor_scalar_max(x2k[:, sl], x2k[:, sl], 0.0)

            # ---------------- attn2 ----------------
            kq2 = ap_.tile([128, NK], BF16, tag="kq2")
            q2 = ap_.tile([128, W], BF16, tag="q2")
            vt2 = ap_.tile([128, JT, 128], BF16, tag="vt2")
            x2a = ap_.tile([128, WP], BF16, tag="x2a")
            nc.vector.memset(x2a[:, 0:PD], 0.0)
            nc.vector.memset(x2a[:, PD + W:WP], 0.0)
            atmp2 = fp.tile([128, 512], F32, tag="atmp2", bufs=2)

            def epi2(ci, cs, cw, avp, rzb):
                nc.vector.tensor_mul(atmp2[:, 0:cw], avp[0:128, 0:cw],
                                     rzb[0:128, 0:cw])
                nc.vector.tensor_add(x2a[:, PD + cs:PD + cs + cw],
                                     atmp2[:, 0:cw], x2q[:, cs:cs + cw])
                if ci == 0:
                    nc.vector.memset(x2a[:, PD:PD + MG], 0.0)

            def mid2(cp_, zr_tile):
                if cp_[0] == 2:
                    nc.sync.dma_start(pw_in[1].ap(), zr_tile[0:64, 0:2])
                    nc.gpsimd.collective_compute(
                        "AllGather", ALU.bypass, replica_groups=ALL8,
                        ins=[pw_in[1].ap().opt()],
                        outs=[pw_out[1].ap().opt()])

            attention(x2q, x2k, O_WK2, O_WQ2, O_WV2, 16, 128, True,
                      kq2, q2, vt2, epi2, mid_hook=mid2)

            # ---------------- conv3 (15 taps, tap-outer) + stats -----------
            c3banksA = sct()
            c3banksB = sct()
            c3small = avt()
            for t in range(15):
                for ci, (cs, cw) in enumerate(CHUNKS):
                    dst = _cbank((c3banksA, c3banksB), c3small, ci)[:, 0:cw]
                    nc.tensor.matmul(dst,
                                     cb[:, O_W3T + 128 * t:O_W3T + 128 * (t + 1)],
                                     x2a[:, PD + cs + t - 7:PD + cs + t - 7 + cw],
                                     start=(t == 0), stop=(t == 14))
            st3 = fp.tile([128, 12], F32, tag="st3")
            for ci, (cs, cw) in enumerate(CHUNKS):
                a, wd = STAT_SL[ci]
                sl = _cbank((c3banksA, c3banksB), c3small,
                            ci)[:, a - cs:a - cs + wd]
                nc.vector.tensor_reduce(st3[:, ci:ci + 1], sl, axis=AX.X,
                                        op=ALU.add)
                nc.scalar.activation(sq_scrA[:, 0:wd], sl, AF.Square,
                                     accum_out=st3[:, 5 + ci:6 + ci])
            stats3 = fp.tile([128, 2], F32, tag="stats3")
            nc.vector.tensor_reduce(stats3[:, 0:1], st3[:, 0:5], axis=AX.X,
                                    op=ALU.add)
            nc.vector.tensor_reduce(stats3[:, 1:2], st3[:, 5:10], axis=AX.X,
                                    op=ALU.add)
            nc.sync.dma_start(ar_in[1].ap(), stats3[:])
            nc.gpsimd.collective_compute(
                "AllGather", ALU.bypass, replica_groups=ALL8,
                ins=[ar_in[1].ap().opt()], outs=[ar_out[1].ap().opt()])
            dve_warm_chain(10, stats3[0:128, 0:2])

            ss3 = bn_from_ar(ar_out[1], C_BN3, "ss3")

            # x3d = relu(bn3(y3)) with dup+shift rows, straight from PSUM
            x3d = ap_.tile([128, WP], BF16, tag="x3d")
            nc.vector.memset(x3d[:, 0:PD], 0.0)
            nc.vector.memset(x3d[:, PD + W - 1:WP], 0.0)
            for ci, (cs, cw) in enumerate(CHUNKS):
                bank = _cbank((c3banksA, c3banksB), c3small, ci)
                src_lo = bank[0:64, 0:cw]
                src_hi = bank[64:128, 0:cw]
                nc.scalar.activation(x3d[0:64, PD + cs:PD + cs + cw], src_lo,
                                     AF.Relu, bias=ss3[0:64, 6:7],
                                     scale=ss3[0:64, 5:6])
                nc.vector.tensor_scalar(x3d[64:128, PD + cs - 1:PD + cs + cw - 1],
                                        src_hi, scalar1=ss3[64:128, 5:6],
                                        scalar2=ss3[64:128, 6:7],
                                        op0=ALU.mult, op1=ALU.add)
                nc.vector.tensor_scalar_max(
                    x3d[64:128, PD + cs - 1:PD + cs + cw - 1],
                    x3d[64:128, PD + cs - 1:PD + cs + cw - 1], 0.0)
            nc.vector.memset(x3d[0:64, PD:PD + MG], 0.0)
            nc.vector.memset(x3d[64:128, PD - 1:PD + MG - 1], 0.0)

            # ---------------- conv4 (tap pairs) + relu, dup+shift ----------
            x4q = ap_.tile([64, WP], BF16, tag="x4q")
            nc.vector.memset(x4q[:, 0:PD], 0.0)
            nc.vector.memset(x4q[:, PD + W - 1:WP], 0.0)
            c4banksA = sct()
            c4banksB = sct()
            c4small = avt()
            for t in range(8):
                for ci, (cs, cw) in enumerate(CHUNKS):
                    dst = _cbank((c4banksA, c4banksB), c4small, ci,
                                 rows=64)[:, 0:cw]
                    nc.tensor.matmul(dst,
                                     cb[:, O_W4P + 64 * t:O_W4P + 64 * (t + 1)],
                                     x3d[:, PD + cs + 2 * t - 7:PD + cs + 2 * t - 7 + cw],
                                     start=(t == 0), stop=(t == 7))
            for ci, (cs, cw) in enumerate(CHUNKS):
                bank4 = _cbank((c4banksA, c4banksB), c4small, ci, rows=64)
                lo = bank4[0:32, 0:cw]
                hi = bank4[32:64, 0:cw]
                nc.scalar.activation(x4q[0:32, PD + cs:PD + cs + cw], lo,
                                     AF.Relu, bias=cf[0:32, C_C4B:C_C4B + 1])
                nc.vector.tensor_scalar(x4q[32:64, PD + cs - 1:PD + cs + cw - 1],
                                        hi, scalar1=cf[32:64, C_C4B:C_C4B + 1],
                                        scalar2=0.0, op0=ALU.add, op1=ALU.max)
            nc.vector.memset(x4q[0:32, PD:PD + MG], 0.0)
            nc.vector.memset(x4q[32:64, PD - 1:PD + MG - 1], 0.0)

            # ---------------- conv5 (tap pairs) + output -------------------
            c5banksA = sct()
            c5banksB = sct()
            for t in range(8):
                for ci, (cs, cw) in enumerate(OUT_CHUNKS):
                    c5b_ = c5banksA if ci < 2 else c5banksB
                    nc.tensor.matmul(c5b_[0:1, ci % 2, 0:cw],
                                     cb[0:64, O_W5P + t:O_W5P + t + 1],
                                     x4q[:, PD + cs + 2 * t - 7:PD + cs + 2 * t - 7 + cw],
                                     start=(t == 0), stop=(t == 7))
            for ci, (cs, cw) in enumerate(OUT_CHUNKS):
                oc = fp.tile([1, 512], F32, tag="oc", bufs=2)
                c5b_ = c5banksA if ci < 2 else c5banksB
                nc.vector.tensor_scalar(oc[:, 0:cw], c5b_[0:1, ci % 2, 0:cw],
                                        scalar1=STRENGTH,
                                        scalar2=STRENGTH * c5b_f,
                                        op0=ALU.mult, op1=ALU.add)
                nc.vector.tensor_add(oc[:, 0:cw], oc[:, 0:cw],
                                     aud[:, cs - MG:cs - MG + cw])
                nc.sync.dma_start(out_d[:, cs - MG:cs - MG + cw], oc[:, 0:cw])

    nc.compile()
    return nc


def _host_prep(audio, message, w1, w2, w3, w4, w5, a1_wq, a1_wk, a1_wv, a1_g,
               a2_wq, a2_wk, a2_wv, a2_g, bn1_g, bn1_b, bn2_g, bn2_b,
               bn3_g, bn3_b, c4b):
    """Build per-core input dicts."""
    in_maps = []
    for core in range(NCORES):
        b, h = core // 2, core % 2
        rev = h == 1

        def fw(w):
            return w[:, :, ::-1] if rev else w

        w1f, w2f, w3f, w4f, w5f = (np.asarray(fw(x), np.float32)
                                   for x in (w1, w2, w3, w4, w5))

        # bf16 const blob
        cbm = np.zeros((128, NB), np.float32)
        # w1c_dup per batch group (group 0 = own orientation, others canonical)
        w1c_own = np.zeros((32, 128), np.float32)
        w1c_can = np.zeros((32, 128), np.float32)
        w1_can = np.asarray(w1, np.float32)
        for t in range(15):
            for ch in range(2):
                w1c_own[2 * t + ch, 0:64] = w1f[:, ch, t]
                w1c_own[2 * t + ch, 64:128] = w1f[:, ch, t]
                w1c_can[2 * t + ch, 0:64] = w1_can[:, ch, t]
                w1c_can[2 * t + ch, 64:128] = w1_can[:, ch, t]
        for g in range(4):
            cbm[32 * g:32 * g + 32, O_W1C:O_W1C + 128] = (w1c_own if g == 0
                                                          else w1c_can)
        # attn1 qk 4x col-replicated, row-duplicated
        wk1T = np.asarray(a1_wk, np.float32).T  # [64, 8]
        wq1T = np.asarray(a1_wq, np.float32).T
        for i in range(4):
            for u in range(2):
                cbm[64 * u:64 * u + 64, O_WK1 + 32 * i:O_WK1 + 32 * i + 8] = wk1T
                cbm[64 * u:64 * u + 64, O_WQ1 + 32 * i:O_WQ1 + 32 * i + 8] = wq1T
        wv1T = np.asarray(a1_wv, np.float32).T * float(np.asarray(a1_g))
        cbm[0:64, O_WV1:O_WV1 + 64] = wv1T
        cbm[64:128, O_WV1:O_WV1 + 64] = wv1T
        # attn2
        wk2T = np.asarray(a2_wk, np.float32).T  # [128, 16]
        wq2T = np.asarray(a2_wq, np.float32).T
        for i in range(4):
            cbm[:, O_WK2 + 32 * i:O_WK2 + 32 * i + 16] = wk2T
            cbm[:, O_WQ2 + 32 * i:O_WQ2 + 32 * i + 16] = wq2T
        cbm[:, O_WV2:O_WV2 + 128] = np.asarray(a2_wv, np.float32).T * float(
            np.asarray(a2_g))
        # conv2 tap pairs [128ch_in x 2 taps, 128 out]
        for t in range(8):
            blk = np.zeros((128, 128), np.float32)
            blk[0:64, :] = w2f[:, :, 2 * t].T
            if 2 * t + 1 < 15:
                blk[64:128, :] = w2f[:, :, 2 * t + 1].T
            cbm[:, O_W2P + 128 * t:O_W2P + 128 * (t + 1)] = blk
        # conv3 taps, out-duplicated
        for t in range(15):
            blk = np.zeros((128, 128), np.float32)
            blk[:, 0:64] = w3f[:, :, t].T
            blk[:, 64:128] = w3f[:, :, t].T
            cbm[:, O_W3T + 128 * t:O_W3T + 128 * (t + 1)] = blk
        # conv4 tap pairs, out-duplicated [64 out]
        for t in range(8):
            blk = np.zeros((128, 64), np.float32)
            blk[0:64, 0:32] = w4f[:, :, 2 * t].T
            blk[0:64, 32:64] = w4f[:, :, 2 * t].T
            if 2 * t + 1 < 15:
                blk[64:128, 0:32] = w4f[:, :, 2 * t + 1].T
                blk[64:128, 32:64] = w4f[:, :, 2 * t + 1].T
            cbm[:, O_W4P + 64 * t:O_W4P + 64 * (t + 1)] = blk
        # conv5 tap pairs [32ch x 2 shifts, 1]
        for t in range(8):
            cbm[0:32, O_W5P + t] = w5f[0, :, 2 * t]
            if 2 * t + 1 < 15:
                cbm[32:64, O_W5P + t] = w5f[0, :, 2 * t + 1]

        # f32 const blob
        cfm = np.zeros((128, NF), np.float32)
        for col, g_, b_ in ((C_BN1, bn1_g, bn1_b), (C_BN3, bn3_g, bn3_b)):
            gg = np.asarray(g_, np.float32); bb = np.asarray(b_, np.float32)
            cfm[0:64, col] = gg; cfm[64:128, col] = gg
            cfm[0:64, col + 1] = bb; cfm[64:128, col + 1] = bb
        cfm[:, C_BN2] = np.asarray(bn2_g, np.float32)
        cfm[:, C_BN2 + 1] = np.asarray(bn2_b, np.float32)
        c4 = np.asarray(c4b, np.float32)
        cfm[0:32, C_C4B] = c4; cfm[32:64, C_C4B] = c4

        # x0 im2col blob: batch group g at partitions 32g (+2t+ch), left-pad 32
        x0m = np.zeros((128, XW), np.float32)
        order = [b] + [x for x in range(B) if x != b]
        for g, bb in enumerate(order):
            arow = np.asarray(audio[bb, 0], np.float32)
            mrow = np.asarray(message[bb, 0], np.float32)
            if g == 0 and rev:
                arow = arow[::-1]; mrow = mrow[::-1]
            # group 0 rows pair with w1c_own (tap-flipped for rev cores),
            # groups 1-3 with w1c_can - handled in the cb blob
            xp = np.zeros((2, L + 96), np.float32)  # pos -48..L+48
            xp[0, 48:48 + L] = arow
            xp[1, 48:48 + L] = mrow
            for t in range(15):
                for ch in range(2):
                    # col c -> pos c-32; im2col val = x[ch, pos + t - 7]
                    # pos+t-7 for c in [0, 4128): range [-39+t, 4089+t)
                    lo = -32 + t - 7
                    x0m[32 * g + 2 * t + ch, 0:4128] = xp[ch, 48 + lo:48 + lo + 4128]
        aud_own = np.asarray(audio[b, 0], np.float32)
        if rev:
            aud_own = aud_own[::-1]
        im = {
            "cb": _bf(cbm),
            "cf": _f32(cfm),
            "x0": _bf(x0m),
            "aud": _f32(aud_own[0:OWN].reshape(1, OWN)),
        }
        in_maps.append(im)
    return in_maps


def kernel(audio, message, w1, c1b, w2, c2b, w3, c3b, w4, c4b, w5, c5b,
           bn1_g, bn1_b, bn2_g, bn2_b, bn3_g, bn3_b,
           a1_wq, a1_wk, a1_wv, a1_g, a2_wq, a2_wk, a2_wv, a2_g,
           _trace=False):
    global LAST_RESULTS
    audio = np.asarray(audio); message = np.asarray(message)
    # c1b/c2b/c3b cancel inside training-mode BatchNorm; c4b/c5b applied.
    in_maps = _host_prep(audio, message, w1, w2, w3, w4, w5,
                         a1_wq, a1_wk, a1_wv, a1_g, a2_wq, a2_wk, a2_wv, a2_g,
                         bn1_g, bn1_b, bn2_g, bn2_b, bn3_g, bn3_b, c4b)
    nc = build_graph(float(np.asarray(c5b).reshape(-1)[0]))
    res = run_bass_kernel_spmd(nc, in_maps, core_ids=list(range(NCORES)),
                               trace=_trace)
    LAST_RESULTS = res

    out = np.zeros((B, 1, L), np.float32)
    for core in range(NCORES):
        b, h = core // 2, core % 2
        o = res.results[core]["out"][0]
        if h == 0:
            out[b, 0, 0:OWN] = o
        else:
            out[b, 0, OWN:L] = o[::-1]
    return out
